# revision 1
# baseline (speedup 1.0000x reference)
"""AFT-Full Trainium2 kernel (8 NeuronCores, SPMD, no collectives).

Math (B=16, C=64, H=W=128, HID=128), from the reference:
    xr  = x.reshape(B, H, W, C)                      # pure reinterpretation
    q/k/v = xr @ w{q,k,v}.T + b{q,k,v}               # per-token projections
    m   = max_b k                                    # batch max, per (h,w,d)
    ek  = exp(k - m)
    num = sum_w ek*v ; den = sum_w ek                # exp_w == 1 identically
    y   = sigmoid(q) * num/den                       # num/den broadcast over w
    out = (y @ wo.T + bo).reshape(B, C, H, W)

Notes exploited:
  * `w` input is unused: exp(w_bias - max(w_bias, axis=0)) == 1 (size-1 axis).
  * bk cancels: k - max_b k is invariant to a per-d constant shift.
  * Nothing crosses H  ->  shard H across the 8 cores; fully local per core.
  * sigmoid(z) = 0.5*tanh(z/2) + 0.5 (tanh shares the ACT `exp` table set).
  * y = sigmoid(q+bq)*(num/den + bv) = halfR*tanh(q/2 + bq/2) + halfR.

Per-core schedule, per h-row (16 per core):
  load x[:, h] as [w=128, b=16, c=64] with f32->bf16 cast in the DMA;
  xbar DMA-transpose sample pairs to xT [c(2 samples on partition halves), w];
  k = wkT.T @ xT (4 matmuls N=512 via tile_position), ACT-copy k to SBUF f32;
  one strided reduce_max gives the batch max; per sample: GPSIMD sub,
  ACT exp with accum_out (den for free), DVE mul ek*v; per group one DVE
  reduce_sum (num); ACT tanh (bf16); DVE tensor_scalar y; out = y.T @ woT
  (4 samples share one PSUM bank + one ACT copy); one DMA per h each way.
"""

import os
import sys
from contextlib import ExitStack

import numpy as np

for _p in ("/opt/trn_rl_repo", "/opt/pypackages"):
    if os.path.isdir(_p) and _p not in sys.path:
        sys.path.append(_p)

import ml_dtypes

import concourse.bacc as bacc
import concourse.bass as bass
import concourse.tile as tile
from concourse import mybir
from concourse.bass_utils import run_bass_kernel_spmd

B, C, H, W, HID = 16, 64, 128, 128, 128
NCORES = 8
HS = H // NCORES  # h-rows per core

F32 = mybir.dt.float32
BF16 = mybir.dt.bfloat16
AX = mybir.AluOpType
ACTF = mybir.ActivationFunctionType


def _build_nc():
    nc = bacc.Bacc(trn_type="TRN2")

    x = nc.dram_tensor("x", [B, HS, W, C], F32, kind="ExternalInput")
    wkqv = nc.dram_tensor("wkqv", [C, 3, HID], BF16, kind="ExternalInput")
    wot = nc.dram_tensor("wot", [HID, C], BF16, kind="ExternalInput")
    wvd = nc.dram_tensor("wvd", [HID, C], F32, kind="ExternalInput")
    hbq = nc.dram_tensor("hbq", [HID, 1], F32, kind="ExternalInput")
    bvv = nc.dram_tensor("bvv", [HID, 1], F32, kind="ExternalInput")
    out = nc.dram_tensor("out", [B, HS, W, C], F32, kind="ExternalOutput")

    with tile.TileContext(nc) as tc, ExitStack() as ctx:
        _body(ctx, tc, x[:], wkqv[:], wot[:], wvd[:], hbq[:], bvv[:], out[:])
    nc.compile()
    return nc


def _body(ctx, tc, x, wkqv, wot, wvd, hbq, bvv, out):
    nc = tc.nc

    xv = x.rearrange("b h w c -> h w b c")  # per-h view [w, b, c]
    ov = out.rearrange("b h w c -> h w b c")

    consts = ctx.enter_context(tc.tile_pool(name="consts", bufs=1))
    p_x = ctx.enter_context(tc.tile_pool(name="xin", bufs=3))
    p_xt = ctx.enter_context(tc.tile_pool(name="xt", bufs=3))
    p_k16 = ctx.enter_context(tc.tile_pool(name="k16", bufs=2))
    p_mt = ctx.enter_context(tc.tile_pool(name="mtree", bufs=2))
    p_ek = ctx.enter_context(tc.tile_pool(name="ek", bufs=2))
    p_s = ctx.enter_context(tc.tile_pool(name="sums", bufs=2))
    p_t = ctx.enter_context(tc.tile_pool(name="tanh", bufs=2))
    p_y = ctx.enter_context(tc.tile_pool(name="y", bufs=3))
    p_o = ctx.enter_context(tc.tile_pool(name="oacc", bufs=2))
    # PSUM budget is 8 banks: pass-1 k (2), pass-2 q/v (4, two groups in
    # flight), output projection (2).
    ps_k = ctx.enter_context(tc.tile_pool(name="ps_k", bufs=2, space="PSUM"))
    ps_qv = ctx.enter_context(tc.tile_pool(name="ps_qv", bufs=4, space="PSUM"))
    ps_o = ctx.enter_context(tc.tile_pool(name="ps_o", bufs=2, space="PSUM"))

    # weights duplicated on both partition halves so b-even (partitions 0:64)
    # and b-odd (64:128) share one stationary layout
    wdup = consts.tile([128, 3, HID], BF16)
    nc.sync.dma_start(out=wdup[0:64], in_=wkqv)
    nc.sync.dma_start(out=wdup[64:128], in_=wkqv)
    wot_sb = consts.tile([HID, C], BF16)
    nc.sync.dma_start(out=wot_sb, in_=wot)
    wv_sb = consts.tile([HID, C], F32)
    nc.sync.dma_start(out=wv_sb, in_=wvd)
    ones_col = consts.tile([128, 1], BF16)
    nc.vector.memset(ones_col, 1.0)
    hbq_sb = consts.tile([HID, 1], F32)
    nc.sync.dma_start(out=hbq_sb, in_=hbq)
    bv_sb = consts.tile([HID, 1], F32)
    nc.sync.dma_start(out=bv_sb, in_=bvv)

    # group g covers samples b = 2*(pr+j) + hi for j in 0..3, with
    # pr = 4*(g%2) pair offset and hi = g//2 the partition half.
    def grp(g):
        hi = g // 2
        pr = 4 * (g % 2)
        par = slice(64 * hi, 64 * hi + 64)
        tp = (64 * hi, 0)
        bs = [2 * (pr + j) + hi for j in range(4)]
        return par, tp, pr, hi, bs

    def bview(t, pr, hi):
        # [128, B, W] tile -> the 4 samples of group (pr, hi): b = 2*b2 + hi
        return t.rearrange("p (b2 two) w -> p b2 two w", two=2)[
            :, pr : pr + 4, hi, :
        ]

    def bview2(t, pr, hi):
        # [128, B] tile -> [128, 4] columns of group (pr, hi)
        return t.rearrange("p (b2 two) -> p b2 two", two=2)[:, pr : pr + 4, hi]

    for h in range(HS):
        x_in = p_x.tile([W, B, C], BF16)
        nc.gpsimd.dma_start(out=x_in, in_=xv[h])  # SWDGE casts f32 -> bf16

        # xbar transpose of sample pairs: [w, 2*64c] -> [2*64c, w]
        xt8 = p_xt.tile([128, 8, W], BF16)
        for p in range(8):
            nc.sync.dma_start(
                out=xt8[:, p, :], in_=x_in[:, 2 * p : 2 * p + 2, :],
                transpose=True,
            )

        # ---- pass 1: ekraw = exp(k) for all 16 samples; batch max of exp ----
        # max_b exp(k) = exp(max_b k), so the batch max can be taken on exp
        # values and the subtraction becomes a multiply by em = 1/max.
        ek16 = p_k16.tile([128, B, W], BF16)
        for g in range(4):
            par, tp, pr, hi, bs = grp(g)
            kp = ps_k.tile([128, 4, W], F32, tag="k")
            for j in range(4):
                nc.tensor.matmul(
                    kp[:, j, :], xt8[par, pr + j, :], wdup[par, 0, :],
                    tile_position=tp,
                )
            nc.scalar.activation(out=bview(ek16, pr, hi), in_=kp, func=ACTF.Exp)
        # batch max as a binary tree of bf16 TT-max ops on DVE (2x mode);
        # Pool's codegen rejects the max ALU op.
        t8 = p_mt.tile([128, 8, W], BF16, tag="t8")
        nc.vector.tensor_max(t8, ek16[:, 0:8, :], ek16[:, 8:16, :])
        t4 = p_mt.tile([128, 4, W], BF16, tag="t4")
        nc.vector.tensor_max(t4, t8[:, 0:4, :], t8[:, 4:8, :])
        t2 = p_mt.tile([128, 2, W], BF16, tag="t2")
        nc.vector.tensor_max(t2, t4[:, 0:2, :], t4[:, 2:4, :])
        mexp = p_mt.tile([128, W], BF16, tag="mh")
        nc.vector.tensor_max(mexp, t2[:, 0, :], t2[:, 1, :])
        emf = p_mt.tile([128, W], F32, tag="emf")
        nc.vector.reciprocal(emf, mexp)
        em = p_mt.tile([128, W], BF16, tag="em")
        nc.gpsimd.tensor_scalar(out=em, in0=emf, scalar1=1.0, scalar2=None,
                                op0=AX.mult)

        # ---- pass 2: q, v; exp/sums/tanh ----
        den = p_s.tile([HID, B], F32, tag="den")
        num = p_s.tile([HID, B], F32, tag="num")
        t16 = p_t.tile([HID, B, W], BF16)
        for g in range(4):
            par, tp, pr, hi, bs = grp(g)
            qp = ps_qv.tile([128, 4, W], F32, tag="qv")
            nc.tensor.matmul(
                qp, wdup[par, 1, :], xt8[par, pr : pr + 4, :], tile_position=tp
            )
            nc.scalar.activation(
                out=bview(t16, pr, hi), in_=qp, func=ACTF.Tanh,
                scale=0.5, bias=hbq_sb,
            )
            ek4 = p_ek.tile([128, 4, W], BF16, tag="ek")
            for j, b in enumerate(bs):
                nc.gpsimd.tensor_mul(ek4[:, j, :], ek16[:, b, :], em)
            # G[d, c] = sum_w ek[w, d] * x[w, c]; col 64 (vs ones) = den
            g4 = ps_qv.tile([128, 4, C + 1], F32, tag="qv")
            for j, b in enumerate(bs):
                nc.tensor.matmul(g4[:, j, 0:C], ek4[:, j, :], x_in[:, b, :])
                nc.tensor.matmul(g4[:, j, C : C + 1], ek4[:, j, :], ones_col)
            gw4 = p_ek.tile([128, 4, C], F32, tag="gw")
            wv_ap = wv_sb[:]
            wv_bc = bass.AP(tensor=wv_ap.tensor, offset=wv_ap.offset,
                            ap=[wv_ap.ap[0], [0, 4], wv_ap.ap[1]])
            nc.vector.tensor_mul(gw4, g4[:, :, 0:C], wv_bc)
            nc.vector.reduce_sum(
                out=bview2(num, pr, hi), in_=gw4, axis=mybir.AxisListType.X
            )
            nc.vector.tensor_copy(bview2(den, pr, hi), g4[:, :, C])

        # halfR = 0.5 * (num/den + bv)   [HID, 16]
        rec = p_s.tile([HID, B], F32, tag="rec")
        nc.vector.reciprocal(rec, den)
        r0 = p_s.tile([HID, B], F32, tag="r0")
        nc.vector.tensor_mul(r0, num, rec)
        halfr = p_s.tile([HID, B], F32, tag="halfr")
        nc.vector.tensor_scalar(
            out=halfr, in0=r0, scalar1=bv_sb, scalar2=0.5,
            op0=AX.add, op1=AX.mult,
        )

        # ---- output projection: 4 samples share one PSUM bank ----
        o16 = p_o.tile([W, B, C], F32)
        for g in range(4):
            par, tp, pr, hi, bs = grp(g)
            op = ps_o.tile([W, 4, C], F32, tag="o")
            for j, b in enumerate(bs):
                y = p_y.tile([HID, W], BF16)
                nc.gpsimd.tensor_scalar(
                    out=y, in0=t16[:, b, :],
                    scalar1=halfr[:, b : b + 1], scalar2=halfr[:, b : b + 1],
                    op0=AX.mult, op1=AX.add,
                )
                nc.tensor.matmul(op[:, j, :], y, wot_sb)
            # copy the 4 samples' outputs in one strided ACT op
            nc.scalar.copy(
                out=o16.rearrange("w (b2 two) c -> w b2 two c", two=2)[
                    :, pr : pr + 4, hi, :
                ],
                in_=op,
            )
        nc.sync.dma_start(out=ov[h], in_=o16)


_NC_CACHE = {}


def _get_nc():
    if "nc" not in _NC_CACHE:
        _NC_CACHE["nc"] = _build_nc()
    return _NC_CACHE["nc"]


def _make_in_maps(inputs):
    x = np.ascontiguousarray(np.asarray(inputs["x"], dtype=np.float32))
    wq = np.asarray(inputs["wq"], np.float32)
    wk = np.asarray(inputs["wk"], np.float32)
    wv = np.asarray(inputs["wv"], np.float32)
    wo = np.asarray(inputs["wo"], np.float32)
    bq = np.asarray(inputs["bq"], np.float32)
    bv = np.asarray(inputs["bv"], np.float32)

    xr = x.reshape(B, H, W, C)
    wkqv = np.ascontiguousarray(
        np.stack([wk.T, wq.T, wv.T], axis=1).astype(ml_dtypes.bfloat16)
    )  # [C, 3, HID]
    wot = np.ascontiguousarray(wo.T.astype(ml_dtypes.bfloat16))  # [HID, C]
    wvd = np.ascontiguousarray(wv)  # [HID, C]
    hbq = np.ascontiguousarray((0.5 * bq).reshape(HID, 1))
    bvv = np.ascontiguousarray(bv.reshape(HID, 1))

    in_maps = []
    for i in range(NCORES):
        in_maps.append(
            {
                "x": np.ascontiguousarray(xr[:, i * HS : (i + 1) * HS]),
                "wkqv": wkqv,
                "wot": wot,
                "wvd": wvd,
                "hbq": hbq,
                "bvv": bvv,
            }
        )
    return in_maps


def run(inputs, trace=False):
    """Run on the 8 NeuronCores; returns (full_output, BassKernelResults)."""
    in_maps = _make_in_maps(inputs)
    res = run_bass_kernel_spmd(
        _get_nc(), in_maps, core_ids=list(range(NCORES)), trace=trace
    )
    shards = [r["out"] for r in res.results]  # each [B, HS, W, C]
    full = np.concatenate(shards, axis=1)  # [B, H, W, C]
    bo = np.asarray(inputs["bo"], np.float32)
    if np.any(bo):
        full = full + bo
    return full.reshape(B, C, H, W).astype(np.float32), res


def kernel(**inputs):
    out, _ = run(inputs, trace=False)
    return out

